# revision 1
# baseline (speedup 1.0000x reference)
"""GAT + GCN + classifier over a COO graph, distributed over 8 TRN2 NeuronCores.

Strategy (dst-sharded message passing):
  - Nodes are dealt round-robin by degree to 8 cores (balances edges/core);
    within a core, nodes are ordered by (in-degree split) and tiled 128 at a
    time so every gather/softmax/reduce is a fixed-shape tile op.
  - Every core builds the full gather table T[n] = [h(n) | a_s(n)] (bf16) from
    x @ Wg_aug on its own PE (replicating this 3.3 GFLOP matmul is cheaper
    than all-gathering 26 MB).
  - GAT edge phase: per dst-tile dma_gather of the padded incoming-edge rows
    (split into lo/hi halves because gather indices are int16), fused
    leaky-relu/softmax on DVE+ACT, weighted sum via a broadcast-AP multiply +
    pairwise tree reduction.
  - GCN phase: per-core u = dinv * (xg @ Wc) (bf16) is AllGathered, then the
    same padded-gather machinery accumulates norm-weighted sums.
  - Classifier + log_softmax in fp32 on-device; host just de-permutes rows.
"""
import sys

sys.path.insert(0, "/opt/trn_rl_repo")

import numpy as np
import ml_dtypes

import concourse.bass as bass
import concourse.bacc as bacc
import concourse.mybir as mybir
import concourse.tile as tile
from concourse.bass_utils import run_bass_kernel_spmd

# problem constants (hardcoded per contract)
N = 50000
E = 800000
F_IN = 128
H = 4
C = 64
HC = H * C          # 256
HID = 128
NCLASS = 10
NEG = 0.2

NCORES = 8
P = 128
NPC = N // NCORES   # 6250 nodes per core
TPC = 49            # tiles per core (49*128 = 6272 >= 6250)
S = TPC * P         # 6272 padded slots per core
SPLIT = 32768       # int16 gather index range per table half
XT = 391            # x tiles for table build (391*128 = 50048)
NPAD = XT * P       # 50048
NT_ROWS = 1 + NPAD + 1   # gather table rows: [dummy | nodes (+pad) | hi dummy]
HI_DUMMY = NT_ROWS - 1   # 50049
TABW = 384          # bf16 table row: 0:256 h, 256:260 a_s, 260:384 junk pad
ASD_NEG = -10000.0  # a_s marker for dummy rows (drives softmax weight to ~0)
NU_ROWS = NCORES * S     # 50176 u-table rows
U_LO_DUMMY = NPC         # row 6250 (core0 pad slot -> always zero)
U_HI_DUMMY = 7 * S + NPC # row 50154 (core7 pad slot)

f32 = mybir.dt.float32
bf16 = mybir.dt.bfloat16
i16 = mybir.dt.int16


def _build_structures(edge_index):
    src = np.asarray(edge_index[0], dtype=np.int64)
    dst = np.asarray(edge_index[1], dtype=np.int64)
    src = np.concatenate([src, np.arange(N, dtype=np.int64)])
    dst = np.concatenate([dst, np.arange(N, dtype=np.int64)])
    deg = np.bincount(dst, minlength=N).astype(np.int64)
    dinv = (1.0 / np.sqrt(deg)).astype(np.float32)

    indptr = np.zeros(N + 1, np.int64)
    np.cumsum(deg, out=indptr[1:])

    # --- GAT split: table row of node n is n+1 ---
    hi1 = (src + 1) >= SPLIT
    d_hi1 = np.bincount(dst[hi1], minlength=N).astype(np.int64)
    d_lo1 = deg - d_hi1
    order1 = np.lexsort((hi1, dst))
    adj1 = src[order1]  # grouped by dst, lo sources first

    # --- node -> core deal (balance edges), then per-core degree sort ---
    by_deg = np.argsort(-deg, kind="stable")
    perm = np.empty((NCORES, NPC), np.int64)
    for c in range(NCORES):
        nodes = by_deg[c::NCORES]
        k = np.lexsort((d_hi1[nodes], d_lo1[nodes]))
        perm[c] = nodes[k]
    pos = np.empty(N, np.int64)
    for c in range(NCORES):
        pos[perm[c]] = c * S + np.arange(NPC)

    # --- GCN split: u-table row of node n is pos[n] ---
    ps = pos[src]
    hi2 = ps >= SPLIT
    d_hi2 = np.bincount(dst[hi2], minlength=N).astype(np.int64)
    d_lo2 = deg - d_hi2
    order2 = np.lexsort((hi2, dst))
    adj2 = ps[order2]  # u-table positions, grouped by dst, lo first

    # --- common (max across cores) per-tile slot profiles ---
    def tile_prof(dvals):
        m = np.zeros((NCORES, S), np.int64)
        for c in range(NCORES):
            m[c, :NPC] = dvals[perm[c]]
        return m.reshape(NCORES, TPC, P).max(axis=(0, 2))

    Dlo = tile_prof(d_lo1)
    Dhi = tile_prof(d_hi1)
    D2lo = tile_prof(d_lo2)
    D2hi = tile_prof(d_hi2)

    def block(nodes, Dt, dcount, base, adj, shift, dummy):
        """Padded [Dt*128] slot-major int index block for one tile."""
        if Dt == 0:
            return np.zeros(0, np.int64)
        nv = np.maximum(nodes, 0)
        cnt = np.where(nodes >= 0, dcount[nv], 0)
        sl = np.arange(Dt)
        ei = base[:, None] + sl[None, :]
        valid = sl[None, :] < cnt[:, None]
        vals = np.where(valid, adj[np.where(valid, ei, 0)] + shift, dummy)
        return vals.T.reshape(-1)  # position = slot*128 + lane

    def wrap16(flat):
        # position i -> [i % 16, i // 16], replicated to 128 partitions
        arr = flat.reshape(-1, 16).T
        return np.tile(arr, (8, 1))

    gat_idx = []
    gcn_idx = []
    for c in range(NCORES):
        nodes_pad = np.full(S, -1, np.int64)
        nodes_pad[:NPC] = perm[c]
        cols1 = []
        cols2 = []
        for t in range(TPC):
            nodes = nodes_pad[t * P:(t + 1) * P]
            nv = np.maximum(nodes, 0)
            b_lo1 = indptr[nv]
            b_hi1 = indptr[nv] + d_lo1[nv]
            b_lo2 = indptr[nv]
            b_hi2 = indptr[nv] + d_lo2[nv]
            lo = block(nodes, Dlo[t], d_lo1, b_lo1, adj1, 1, 0)
            hi = block(nodes, Dhi[t], d_hi1, b_hi1, adj1, 1 - SPLIT, HI_DUMMY - SPLIT)
            assert lo.size == 0 or (0 <= lo.min() and lo.max() < SPLIT)
            assert hi.size == 0 or (0 <= hi.min() and hi.max() <= HI_DUMMY - SPLIT)
            cols1.append(wrap16(lo))
            cols1.append(wrap16(hi))
            lo2 = block(nodes, D2lo[t], d_lo2, b_lo2, adj2, 0, U_LO_DUMMY)
            hi2b = block(nodes, D2hi[t], d_hi2, b_hi2, adj2, -SPLIT, U_HI_DUMMY - SPLIT)
            assert lo2.size == 0 or (0 <= lo2.min() and lo2.max() < SPLIT)
            assert hi2b.size == 0 or (0 <= hi2b.min() and hi2b.max() < SPLIT)
            cols2.append(wrap16(lo2))
            cols2.append(wrap16(hi2b))
        gat_idx.append(np.concatenate(cols1, axis=1).astype(np.int16))
        gcn_idx.append(np.concatenate(cols2, axis=1).astype(np.int16))

    return dict(
        dinv=dinv, perm=perm,
        Dlo=Dlo.tolist(), Dhi=Dhi.tolist(),
        D2lo=D2lo.tolist(), D2hi=D2hi.tolist(),
        gat_idx=gat_idx, gcn_idx=gcn_idx,
    )


def _build_kernel(Dlo, Dhi, D2lo, D2hi, gat_cols, gcn_cols, phases='ABCDEF'):
    nc = bacc.Bacc(None, num_devices=NCORES)

    x_pad = nc.declare_dram_parameter("x_pad", [NPAD, F_IN], bf16, isOutput=False)
    x_perm = nc.declare_dram_parameter("x_perm", [S, F_IN], bf16, isOutput=False)
    dinv_pt = nc.declare_dram_parameter("dinv_pt", [P, TPC], f32, isOutput=False)
    gat_idx = nc.declare_dram_parameter("gat_idx", [P, gat_cols], i16, isOutput=False)
    gcn_idx = nc.declare_dram_parameter("gcn_idx", [P, gcn_cols], i16, isOutput=False)
    wg_aug = nc.declare_dram_parameter("wg_aug", [F_IN, TABW], bf16, isOutput=False)
    wg_ad = nc.declare_dram_parameter("wg_ad", [F_IN, H], bf16, isOutput=False)
    wc = nc.declare_dram_parameter("wc", [HC, HID], bf16, isOutput=False)
    wl = nc.declare_dram_parameter("wl", [HID, NCLASS], f32, isOutput=False)
    bg_b = nc.declare_dram_parameter("bg_b", [P, HC], f32, isOutput=False)
    bc_b = nc.declare_dram_parameter("bc_b", [P, HID], f32, isOutput=False)
    bl_b = nc.declare_dram_parameter("bl_b", [P, NCLASS], f32, isOutput=False)
    ident_bf_in = nc.declare_dram_parameter("ident_bf", [P, P], bf16, isOutput=False)
    ident_f_in = nc.declare_dram_parameter("ident_f", [P, P], f32, isOutput=False)
    out = nc.declare_dram_parameter("out", [S, NCLASS], f32, isOutput=True)

    h_table = nc.dram_tensor("h_table", [NT_ROWS, TABW], bf16)
    ag_in = nc.dram_tensor("ag_in", [S, HID], bf16)
    ag_out = nc.dram_tensor("ag_out", [NU_ROWS, HID], bf16, addr_space="Shared")

    AW = TABW  # full table width so every gathered byte is initialized

    with tile.TileContext(nc) as tc:
        with (
            tc.tile_pool(name="const", bufs=1) as cpool,
            tc.tile_pool(name="sbuf", bufs=3) as sb,
            tc.tile_pool(name="gat", bufs=2) as gp,
            tc.tile_pool(name="scratch", bufs=1) as sp,
            tc.tile_pool(name="psum", bufs=2, space="PSUM") as pp,
            tc.tile_pool(name="psum1", bufs=2, space="PSUM") as pp1,
        ):
            # ---- resident constants ----
            ident_bf = cpool.tile([P, P], bf16)
            nc.sync.dma_start(out=ident_bf[:], in_=ident_bf_in[:])
            ident_f = cpool.tile([P, P], f32)
            nc.sync.dma_start(out=ident_f[:], in_=ident_f_in[:])
            wga_t = cpool.tile([F_IN, AW], bf16)
            nc.sync.dma_start(out=wga_t[:], in_=wg_aug[:])
            wgad_t = cpool.tile([F_IN, H], bf16)
            nc.sync.dma_start(out=wgad_t[:], in_=wg_ad[:])
            wc_t = cpool.tile([P, 2, HID], bf16)
            nc.sync.dma_start(out=wc_t[:], in_=wc.rearrange("(k p) n -> p k n", p=P))
            wl_t = cpool.tile([HID, NCLASS], f32)
            nc.sync.dma_start(out=wl_t[:], in_=wl[:])
            bg_t = cpool.tile([P, HC], f32)
            nc.sync.dma_start(out=bg_t[:], in_=bg_b[:])
            bc_t = cpool.tile([P, HID], f32)
            nc.sync.dma_start(out=bc_t[:], in_=bc_b[:])
            bl_t = cpool.tile([P, NCLASS], f32)
            nc.sync.dma_start(out=bl_t[:], in_=bl_b[:])
            dinv_t = cpool.tile([P, TPC], f32)
            nc.sync.dma_start(out=dinv_t[:], in_=dinv_pt[:])
            ad_all = cpool.tile([P, TPC * H], f32)

            # ---- dummy table rows ----
            import os as _os2
            _nodum = _os2.environ.get("NB_NODUM")
            _noa2 = _os2.environ.get("NB_NOA2")
            _xt = int(_os2.environ.get("NB_XT", XT))
            dum = cpool.tile([1, TABW], bf16)
            nc.vector.memset(dum[:], 0.0)
            nc.vector.memset(dum[:, HC:HC + H], ASD_NEG)
            if not _nodum:
                nc.sync.dma_start(out=h_table[0:1, :], in_=dum[:])
                nc.sync.dma_start(out=h_table[HI_DUMMY:HI_DUMMY + 1, :], in_=dum[:])

            # ---- phase A: build gather table rows 1..NPAD ----
            for i in range(min(XT, _xt) if 'A' in phases else 0):
                xt = sb.tile([P, F_IN], bf16, tag="xa")
                nc.sync.dma_start(out=xt[:], in_=x_pad[i * P:(i + 1) * P, :])
                xT_ps = pp.tile([P, P], bf16, tag="tr_ps")
                nc.tensor.transpose(xT_ps[:], xt[:], ident_bf[:])
                xT = sb.tile([P, P], bf16, tag="xT")
                nc.vector.tensor_copy(out=xT[:], in_=xT_ps[:])
                hps = pp1.tile([P, AW], f32, tag="mm_ps")
                nc.tensor.matmul(hps[:], lhsT=xT[:], rhs=wga_t[:], start=True, stop=True)
                hbf = sb.tile([P, AW], bf16, tag="hbf")
                nc.vector.tensor_copy(out=hbf[:], in_=hps[:])
                nc.sync.dma_start(
                    out=h_table[1 + i * P:1 + (i + 1) * P, :], in_=hbf[:]
                )

            # ---- phase A2: per-tile a_d for this core's own nodes ----
            for t in range(TPC if ('A' in phases and not _noa2) else 0):
                xt = sb.tile([P, F_IN], bf16, tag="xa")
                nc.sync.dma_start(out=xt[:], in_=x_perm[t * P:(t + 1) * P, :])
                xT_ps = pp.tile([P, P], bf16, tag="tr_ps")
                nc.tensor.transpose(xT_ps[:], xt[:], ident_bf[:])
                xT = sb.tile([P, P], bf16, tag="xT")
                nc.vector.tensor_copy(out=xT[:], in_=xT_ps[:])
                adps = pp1.tile([P, H], f32, tag="mm_ps")
                nc.tensor.matmul(adps[:], lhsT=xT[:], rhs=wgad_t[:], start=True, stop=True)
                nc.vector.tensor_copy(out=ad_all[:, t * H:(t + 1) * H], in_=adps[:])

            # ---- phase B+C: GAT per tile, fused GCN u production ----
            goff = 0
            import os as _os
            _nb = int(_os.environ.get("NB_TILES", TPC))
            _stage = int(_os.environ.get("NB_STAGE", 99))
            for t in range(min(TPC, _nb) if 'B' in phases else 0):
                dlo, dhi = Dlo[t], Dhi[t]
                D = dlo + dhi
                w = 8 * D
                idx_t = gp.tile([P, w], i16, tag="gidx")
                nc.sync.dma_start(out=idx_t[:], in_=gat_idx[:, goff:goff + w])
                goff += w
                G = gp.tile([P, D, TABW], bf16, tag="G")
                if _stage < 0:
                    continue
                if dlo > 0:
                    nc.gpsimd.dma_gather(
                        out_ap=G[:, 0:dlo, :],
                        in_ap=h_table[:, :],
                        idxs_ap=idx_t[:, 0:8 * dlo],
                        num_idxs=P * dlo,
                        num_idxs_reg=P * dlo,
                        elem_size=TABW,
                        single_packet=False,
                    )
                if dhi > 0:
                    nc.gpsimd.dma_gather(
                        out_ap=G[:, dlo:D, :],
                        in_ap=h_table[SPLIT:, :],
                        idxs_ap=idx_t[:, 8 * dlo:w],
                        num_idxs=P * dhi,
                        num_idxs_reg=P * dhi,
                        elem_size=TABW,
                        single_packet=False,
                    )
                if _stage < 1:
                    continue
                # e = leaky_relu(a_s[src] + a_d[dst])
                e = sp.tile([P, H, D], f32, tag="e")
                for h in range(H):
                    nc.vector.tensor_scalar(
                        out=e[:, h, :], in0=G[:, :, HC + h],
                        scalar1=ad_all[:, t * H + h:t * H + h + 1], scalar2=None,
                        op0=mybir.AluOpType.add,
                    )
                if _stage < 2:
                    continue
                e2 = sp.tile([P, H, D], f32, tag="e2")
                nc.vector.tensor_scalar(
                    out=e2[:], in0=e[:], scalar1=NEG, scalar2=None,
                    op0=mybir.AluOpType.mult,
                )
                nc.vector.tensor_tensor(
                    out=e2[:], in0=e[:], in1=e2[:], op=mybir.AluOpType.max
                )
                negm = gp.tile([P, H], f32, tag="negm")
                nc.vector.tensor_reduce(
                    out=negm[:], in_=e2[:], axis=mybir.AxisListType.X,
                    op=mybir.AluOpType.max, negate=True,
                )
                ex = sp.tile([P, H, D], f32, tag="ex")
                den = gp.tile([P, H], f32, tag="den")
                for h in range(H):
                    nc.vector.tensor_scalar(
                        out=ex[:, h, :], in0=e2[:, h, :],
                        scalar1=negm[:, h:h + 1], scalar2=-80.0,
                        op0=mybir.AluOpType.add, op1=mybir.AluOpType.max,
                    )
                    nc.scalar.activation(
                        out=ex[:, h, :], in_=ex[:, h, :],
                        func=mybir.ActivationFunctionType.Exp,
                        accum_out=den[:, h:h + 1],
                    )
                if _stage < 3:
                    continue
                rden = gp.tile([P, H], f32, tag="rden")
                nc.vector.reciprocal(rden[:], den[:])
                # prod[p, d, h, c] = h_gathered * ex  (bf16)
                prod = sp.tile([P, D, HC], bf16, tag="prod")
                g_h = G[:, :, 0:HC].rearrange("p d (h c) -> p d h c", h=H)
                ex_b = ex.rearrange("p h d -> p d h")[:, :, :, None].to_broadcast(
                    [P, D, H, C]
                )
                nc.vector.tensor_tensor(
                    out=prod.rearrange("p d (h c) -> p d h c", h=H),
                    in0=g_h, in1=ex_b, op=mybir.AluOpType.mult,
                )
                if _stage < 4:
                    continue
                # tree-reduce over D slots -> acc f32 [P, HC]
                if D == 1:
                    acc = sp.tile([P, 1, HC], f32, tag="accT")
                    nc.vector.tensor_copy(out=acc[:, 0, :], in_=prod[:, 0, :])
                else:
                    half = D // 2
                    acc = sp.tile([P, max(half, 1), HC], f32, tag="accT")
                    nc.vector.tensor_tensor(
                        out=acc[:, 0:half, :], in0=prod[:, 0:half, :],
                        in1=prod[:, half:2 * half, :], op=mybir.AluOpType.add,
                    )
                    if D % 2:
                        nc.vector.tensor_tensor(
                            out=acc[:, 0, :], in0=acc[:, 0, :],
                            in1=prod[:, 2 * half, :], op=mybir.AluOpType.add,
                        )
                    cur = half
                    while cur > 1:
                        h2 = cur // 2
                        nc.vector.tensor_tensor(
                            out=acc[:, 0:h2, :], in0=acc[:, 0:h2, :],
                            in1=acc[:, h2:2 * h2, :], op=mybir.AluOpType.add,
                        )
                        if cur % 2:
                            nc.vector.tensor_tensor(
                                out=acc[:, 0, :], in0=acc[:, 0, :],
                                in1=acc[:, 2 * h2, :], op=mybir.AluOpType.add,
                            )
                        cur = h2
                if _stage < 5:
                    continue
                # xg = elu(acc / den + bg)
                xg = gp.tile([P, HC], f32, tag="xg")
                accv = acc[:, 0, :].rearrange("p (h c) -> p h c", h=H)
                for h in range(H):
                    nc.vector.tensor_scalar(
                        out=xg[:, h * C:(h + 1) * C], in0=accv[:, h, :],
                        scalar1=rden[:, h:h + 1], scalar2=None,
                        op0=mybir.AluOpType.mult,
                    )
                nc.vector.tensor_tensor(
                    out=xg[:], in0=xg[:], in1=bg_t[:], op=mybir.AluOpType.add
                )
                xneg = gp.tile([P, HC], f32, tag="xneg")
                nc.vector.tensor_scalar(
                    out=xneg[:], in0=xg[:], scalar1=0.0, scalar2=None,
                    op0=mybir.AluOpType.min,
                )
                eexp = gp.tile([P, HC], f32, tag="eexp")
                nc.scalar.activation(
                    out=eexp[:], in_=xneg[:], func=mybir.ActivationFunctionType.Exp
                )
                nc.vector.tensor_scalar(
                    out=eexp[:], in0=eexp[:], scalar1=1.0, scalar2=None,
                    op0=mybir.AluOpType.subtract,
                )
                nc.vector.tensor_scalar(
                    out=xg[:], in0=xg[:], scalar1=0.0, scalar2=None,
                    op0=mybir.AluOpType.max,
                )
                nc.vector.tensor_tensor(
                    out=xg[:], in0=xg[:], in1=eexp[:], op=mybir.AluOpType.add
                )
                if _stage < 6:
                    continue
                # phase C: u = dinv * (xg @ Wc), bf16
                xgb = gp.tile([P, HC], bf16, tag="xgb")
                nc.vector.tensor_copy(out=xgb[:], in_=xg[:])
                xwps = pp1.tile([P, HID], f32, tag="mm_ps")
                for k in range(2):
                    xgT_ps = pp.tile([P, P], bf16, tag="tr_ps")
                    nc.tensor.transpose(
                        xgT_ps[:], xgb[:, k * P:(k + 1) * P], ident_bf[:]
                    )
                    xgT = sb.tile([P, P], bf16, tag="xT")
                    nc.vector.tensor_copy(out=xgT[:], in_=xgT_ps[:])
                    nc.tensor.matmul(
                        xwps[:], lhsT=xgT[:], rhs=wc_t[:, k, :],
                        start=(k == 0), stop=(k == 1),
                    )
                ub = gp.tile([P, HID], bf16, tag="ub")
                nc.vector.tensor_scalar(
                    out=ub[:], in0=xwps[:], scalar1=dinv_t[:, t:t + 1], scalar2=None,
                    op0=mybir.AluOpType.mult,
                )
                nc.sync.dma_start(out=ag_in[t * P:(t + 1) * P, :], in_=ub[:])

            # ---- phase D: AllGather u across cores ----
            if 'D' in phases:
              nc.gpsimd.collective_compute(
                "AllGather",
                mybir.AluOpType.bypass,
                replica_groups=[list(range(NCORES))],
                ins=[ag_in[:]],
                outs=[ag_out[:]],
              )

            # ---- phase E+F: GCN + classifier per tile ----
            goff = 0
            for t in range(TPC if 'E' in phases else 0):
                dlo, dhi = D2lo[t], D2hi[t]
                D = dlo + dhi
                w = 8 * D
                idx_t = gp.tile([P, w], i16, tag="gidx2")
                nc.sync.dma_start(out=idx_t[:], in_=gcn_idx[:, goff:goff + w])
                goff += w
                Gu = gp.tile([P, D, HID], bf16, tag="Gu")
                if dlo > 0:
                    nc.gpsimd.dma_gather(
                        out_ap=Gu[:, 0:dlo, :],
                        in_ap=ag_out[:, :],
                        idxs_ap=idx_t[:, 0:8 * dlo],
                        num_idxs=P * dlo,
                        num_idxs_reg=P * dlo,
                        elem_size=HID,
                        single_packet=False,
                    )
                if dhi > 0:
                    nc.gpsimd.dma_gather(
                        out_ap=Gu[:, dlo:D, :],
                        in_ap=ag_out[SPLIT:, :],
                        idxs_ap=idx_t[:, 8 * dlo:w],
                        num_idxs=P * dhi,
                        num_idxs_reg=P * dhi,
                        elem_size=HID,
                        single_packet=False,
                    )
                if D == 1:
                    uacc = sp.tile([P, 1, HID], f32, tag="uacc")
                    nc.vector.tensor_copy(out=uacc[:, 0, :], in_=Gu[:, 0, :])
                else:
                    half = D // 2
                    uacc = sp.tile([P, max(half, 1), HID], f32, tag="uacc")
                    nc.vector.tensor_tensor(
                        out=uacc[:, 0:half, :], in0=Gu[:, 0:half, :],
                        in1=Gu[:, half:2 * half, :], op=mybir.AluOpType.add,
                    )
                    if D % 2:
                        nc.vector.tensor_tensor(
                            out=uacc[:, 0, :], in0=uacc[:, 0, :],
                            in1=Gu[:, 2 * half, :], op=mybir.AluOpType.add,
                        )
                    cur = half
                    while cur > 1:
                        h2 = cur // 2
                        nc.vector.tensor_tensor(
                            out=uacc[:, 0:h2, :], in0=uacc[:, 0:h2, :],
                            in1=uacc[:, h2:2 * h2, :], op=mybir.AluOpType.add,
                        )
                        if cur % 2:
                            nc.vector.tensor_tensor(
                                out=uacc[:, 0, :], in0=uacc[:, 0, :],
                                in1=uacc[:, 2 * h2, :], op=mybir.AluOpType.add,
                            )
                        cur = h2
                # xc = relu(dinv * sum + bc)
                xc = gp.tile([P, HID], f32, tag="xc")
                nc.vector.tensor_scalar(
                    out=xc[:], in0=uacc[:, 0, :], scalar1=dinv_t[:, t:t + 1],
                    scalar2=None, op0=mybir.AluOpType.mult,
                )
                nc.vector.tensor_tensor(
                    out=xc[:], in0=xc[:], in1=bc_t[:], op=mybir.AluOpType.add
                )
                nc.vector.tensor_scalar(
                    out=xc[:], in0=xc[:], scalar1=0.0, scalar2=None,
                    op0=mybir.AluOpType.max,
                )
                # classifier (fp32)
                xcT_ps = pp.tile([P, P], f32, tag="tr_ps")
                nc.tensor.transpose(xcT_ps[:], xc[:], ident_f[:])
                xcT = sb.tile([P, P], f32, tag="xcT")
                nc.vector.tensor_copy(out=xcT[:], in_=xcT_ps[:])
                lps = pp1.tile([P, NCLASS], f32, tag="mm_ps")
                nc.tensor.matmul(lps[:], lhsT=xcT[:], rhs=wl_t[:], start=True, stop=True)
                lg = gp.tile([P, NCLASS], f32, tag="lg")
                nc.vector.tensor_tensor(
                    out=lg[:], in0=lps[:], in1=bl_t[:], op=mybir.AluOpType.add
                )
                nmx = gp.tile([P, 1], f32, tag="nmx")
                nc.vector.tensor_reduce(
                    out=nmx[:], in_=lg[:], axis=mybir.AxisListType.X,
                    op=mybir.AluOpType.max, negate=True,
                )
                nc.vector.tensor_scalar(
                    out=lg[:], in0=lg[:], scalar1=nmx[:, 0:1], scalar2=None,
                    op0=mybir.AluOpType.add,
                )
                exl = gp.tile([P, NCLASS], f32, tag="exl")
                sume = sp.tile([P, 1], f32, tag="sume")
                nc.scalar.activation(
                    out=exl[:], in_=lg[:], func=mybir.ActivationFunctionType.Exp,
                    accum_out=sume[:, 0:1],
                )
                lns = gp.tile([P, 1], f32, tag="lns")
                nc.scalar.activation(
                    out=lns[:], in_=sume[:], func=mybir.ActivationFunctionType.Ln
                )
                ot = gp.tile([P, NCLASS], f32, tag="ot")
                nc.vector.tensor_scalar(
                    out=ot[:], in0=lg[:], scalar1=lns[:, 0:1], scalar2=None,
                    op0=mybir.AluOpType.subtract,
                )
                nc.sync.dma_start(out=out[t * P:(t + 1) * P, :], in_=ot[:])

    nc.compile()
    return nc


def _prepare(inputs):
    x = np.asarray(inputs["x"], np.float32)
    Wg = np.asarray(inputs["Wg"], np.float32)
    att_src = np.asarray(inputs["att_src"], np.float32)
    att_dst = np.asarray(inputs["att_dst"], np.float32)
    bg = np.asarray(inputs["bg"], np.float32)
    Wc = np.asarray(inputs["Wc"], np.float32)
    bc = np.asarray(inputs["bc"], np.float32)
    Wl = np.asarray(inputs["Wl"], np.float32)
    bl = np.asarray(inputs["bl"], np.float32)
    edge_index = np.asarray(inputs["edge_index"])

    st = _build_structures(edge_index)

    # fold attention vectors into the feature matmul: a_s = x @ (Wg @ As)
    As = np.zeros((HC, H), np.float32)
    Ad = np.zeros((HC, H), np.float32)
    for h in range(H):
        As[h * C:(h + 1) * C, h] = att_src[h]
        Ad[h * C:(h + 1) * C, h] = att_dst[h]
    wg_aug = np.concatenate(
        [Wg, Wg @ As, np.zeros((F_IN, TABW - HC - H), np.float32)], axis=1
    )  # [128, 384], zero-padded so phase A initializes full table rows
    wg_ad = Wg @ Ad                                 # [128, 4]

    x_pad = np.zeros((NPAD, F_IN), np.float32)
    x_pad[:N] = x

    bf = ml_dtypes.bfloat16
    in_maps = []
    for c in range(NCORES):
        xp = np.zeros((S, F_IN), np.float32)
        xp[:NPC] = x[st["perm"][c]]
        dv = np.zeros((P, TPC), np.float32)
        dvp = np.zeros(S, np.float32)
        dvp[:NPC] = st["dinv"][st["perm"][c]]
        dv[:, :] = dvp.reshape(TPC, P).T
        in_maps.append({
            "x_pad": x_pad.astype(bf),
            "x_perm": xp.astype(bf),
            "dinv_pt": dv,
            "gat_idx": st["gat_idx"][c],
            "gcn_idx": st["gcn_idx"][c],
            "wg_aug": wg_aug.astype(bf),
            "wg_ad": wg_ad.astype(bf),
            "wc": Wc.astype(bf),
            "wl": Wl,
            "bg_b": np.tile(bg[None, :], (P, 1)),
            "bc_b": np.tile(bc[None, :], (P, 1)),
            "bl_b": np.tile(bl[None, :], (P, 1)),
            "ident_bf": np.eye(P, dtype=bf),
            "ident_f": np.eye(P, dtype=np.float32),
        })
    return st, in_maps


def _run(inputs, trace=False, trace_kwargs=None, phases='ABCDEF'):
    st, in_maps = _prepare(inputs)
    nc = _build_kernel(
        st["Dlo"], st["Dhi"], st["D2lo"], st["D2hi"],
        st["gat_idx"][0].shape[1], st["gcn_idx"][0].shape[1], phases=phases,
    )
    res = run_bass_kernel_spmd(
        nc, in_maps, list(range(NCORES)), trace=trace, **(trace_kwargs or {})
    )
    out = np.empty((N, NCLASS), np.float32)
    for c in range(NCORES):
        out[st["perm"][c]] = res.results[c]["out"][:NPC]
    return out, res


def kernel(**inputs) -> np.ndarray:
    out, _ = _run(inputs, trace=False)
    return out



# revision 7
# speedup vs baseline: 1.3010x; 1.3010x over previous
"""GAT + GCN + classifier over a COO graph, distributed over 8 TRN2 NeuronCores.

v2 strategy (dst-sharded message passing, shared pos-ordered tables):
  - Nodes are dealt to 8 cores by degree, then re-dealt within the lo group
    (cores 0-4) and hi group (cores 5-7) so both phases' gather tables share
    ONE row order: table row of a node is its slot position `pos`.  The int16
    gather-index split is at row 31360 (= core 5 start), so lo/hi membership
    is "source core < 5" for BOTH the GAT h-table and the GCN u-table --
    one index array serves both phases.
  - Within a core, nodes are sorted by (d_lo, d_hi) with a 4-tile windowed
    re-sort by d_hi: per-tile padded slot count is ~1.25x the true edge count.
  - x arrives pre-transposed and pos-ordered; phase A is a pure
    load->matmul->cast->store pipeline (batched 4 tiles) building the bf16
    h-table [row = h(256) | a_s(4) | pad], plus per-core a_d columns.
  - GAT per dst tile: 2 dma_gathers (lo/hi), softmax fused as a handful of
    whole-tile strided-AP vector ops, weighted sum via broadcast multiply +
    pairwise tree reduce, ELU, then u = dinv*(xg @ Wc) via DMA-transpose +
    matmul, stored to ag_in.
  - AllGather u across cores; GCN per tile: 2 dma_gathers from the u table,
    tree reduce, relu epilogue, classifier matmul batched 4 tiles per PSUM
    group with one fused log_softmax chain per group.
"""
import sys

sys.path.insert(0, "/opt/trn_rl_repo")

import numpy as np
import ml_dtypes

import concourse.bass as bass
import concourse.bacc as bacc
import concourse.mybir as mybir
import concourse.tile as tile
from concourse.bass_utils import run_bass_kernel_spmd

# problem constants (hardcoded per contract)
N = 50000
E = 800000
F_IN = 128
H = 4
C = 64
HC = H * C          # 256
HID = 128
NCLASS = 10
NEG = 0.2

NCORES = 8
P = 128
NPC = N // NCORES   # 6250 real nodes per core
TPC = 49            # tiles per core
S = TPC * P         # 6272 slots per core
NU = NCORES * S     # 50176 table rows (pos space)
NLO = 5             # cores 0..4 are the "lo" half
SPLIT_AT = NLO * S  # 31360; hi rows span [31360, 50176) -> fits int16 shifted
TABW = 384          # h-table row: 0:256 h | 256:260 a_s | 260:384 zero pad
ASD_NEG = -10000.0
LO_DUMMY = 6271           # pos of a core-0 pad slot (< SPLIT_AT)
HI_DUMMY = NU - 1         # pos of a core-7 pad slot (>= SPLIT_AT)

f32 = mybir.dt.float32
bf16 = mybir.dt.bfloat16
i16 = mybir.dt.int16

FGRP = 4            # classifier tiles per PSUM/log-softmax group


def _build_structures(edge_index):
    src = np.asarray(edge_index[0], dtype=np.int64)
    dst = np.asarray(edge_index[1], dtype=np.int64)
    src = np.concatenate([src, np.arange(N, dtype=np.int64)])
    dst = np.concatenate([dst, np.arange(N, dtype=np.int64)])
    deg = np.bincount(dst, minlength=N).astype(np.int64)
    dinv = (1.0 / np.sqrt(deg)).astype(np.float32)

    # --- deal pass 1: by degree ---
    order = np.argsort(-deg, kind="stable")
    perm = np.stack([order[c::NCORES] for c in range(NCORES)])

    # membership: hi iff source core >= NLO
    core_of = np.empty(N, np.int64)
    for c in range(NCORES):
        core_of[perm[c]] = c
    hi_node = core_of >= NLO
    d_hi = np.bincount(dst[hi_node[src]], minlength=N).astype(np.int64)
    d_lo = deg - d_hi

    # --- deal pass 2 within groups (preserves membership) ---
    key = d_lo * 1000 + d_hi
    lo_nodes = np.where(~hi_node)[0]
    hi_nodes = np.where(hi_node)[0]
    lo_sorted = lo_nodes[np.argsort(-key[lo_nodes], kind="stable")]
    hi_sorted = hi_nodes[np.argsort(-key[hi_nodes], kind="stable")]
    perm = np.empty((NCORES, NPC), np.int64)
    for c in range(NLO):
        perm[c] = lo_sorted[c::NLO]
    for c in range(NCORES - NLO):
        perm[NLO + c] = hi_sorted[c::NCORES - NLO]

    # --- within-core sort: (d_lo, d_hi) then 4-tile window re-sort by d_hi ---
    W = 4 * P
    for c in range(NCORES):
        nodes = perm[c]
        nodes = nodes[np.argsort(-key[nodes], kind="stable")]
        blocks = []
        for s0 in range(0, NPC, W):
            blk = nodes[s0:s0 + W]
            blocks.append(blk[np.argsort(-d_hi[blk], kind="stable")])
        perm[c] = np.concatenate(blocks)

    pos = np.full(N, -1, np.int64)
    for c in range(NCORES):
        pos[perm[c]] = c * S + np.arange(NPC)

    # --- per-tile profiles (max over cores and lanes) ---
    mlo = np.zeros((NCORES, S), np.int64)
    mhi = np.zeros((NCORES, S), np.int64)
    for c in range(NCORES):
        mlo[c, :NPC] = d_lo[perm[c]]
        mhi[c, :NPC] = d_hi[perm[c]]
    Dlo = mlo.reshape(NCORES, TPC, P).max(axis=(0, 2))
    Dhi = mhi.reshape(NCORES, TPC, P).max(axis=(0, 2))

    # --- adjacency grouped by dst, lo sources first; values are pos[src] ---
    hi_e = hi_node[src]
    order_e = np.lexsort((hi_e, dst))
    adj = pos[src[order_e]]
    indptr = np.zeros(N + 1, np.int64)
    np.cumsum(deg, out=indptr[1:])

    def block(nodes, Dt, dcount, base, shift, dummy):
        if Dt == 0:
            return np.zeros(0, np.int64)
        nv = np.maximum(nodes, 0)
        cnt = np.where(nodes >= 0, dcount[nv], 0)
        sl = np.arange(Dt)
        ei = base[:, None] + sl[None, :]
        valid = sl[None, :] < cnt[:, None]
        vals = np.where(valid, adj[np.where(valid, ei, 0)] + shift, dummy)
        return vals.T.reshape(-1)  # position = slot*128 + lane

    def wrap16(flat):
        arr = flat.reshape(-1, 16).T
        return np.tile(arr, (8, 1))

    idx_maps = []
    for c in range(NCORES):
        nodes_pad = np.full(S, -1, np.int64)
        nodes_pad[:NPC] = perm[c]
        cols = []
        for t in range(TPC):
            nodes = nodes_pad[t * P:(t + 1) * P]
            nv = np.maximum(nodes, 0)
            b_lo = indptr[nv]
            b_hi = indptr[nv] + d_lo[nv]
            lo = block(nodes, Dlo[t], d_lo, b_lo, 0, LO_DUMMY)
            hi = block(nodes, Dhi[t], d_hi, b_hi, -SPLIT_AT, HI_DUMMY - SPLIT_AT)
            assert lo.size == 0 or (0 <= lo.min() and lo.max() < 32768)
            assert hi.size == 0 or (0 <= hi.min() and hi.max() < 32768)
            cols.append(wrap16(lo))
            cols.append(wrap16(hi))
        idx_maps.append(np.concatenate(cols, axis=1).astype(np.int16))

    return dict(
        dinv=dinv, perm=perm, pos=pos,
        Dlo=Dlo.tolist(), Dhi=Dhi.tolist(), idx=idx_maps,
    )


def _tree_reduce(nc, sp, prod, D, width, tag):
    """Pairwise tree sum over the slot axis of prod [P, D, width] -> [P, width] f32."""
    if D == 1:
        acc = sp.tile([P, 1, width], f32, tag=tag)
        nc.vector.tensor_copy(out=acc[:, 0, :], in_=prod[:, 0, :])
        return acc
    half = D // 2
    acc = sp.tile([P, max(half, 1), width], f32, tag=tag)
    nc.vector.tensor_tensor(
        out=acc[:, 0:half, :], in0=prod[:, 0:half, :],
        in1=prod[:, half:2 * half, :], op=mybir.AluOpType.add,
    )
    if D % 2:
        nc.vector.tensor_tensor(
            out=acc[:, 0, :], in0=acc[:, 0, :],
            in1=prod[:, 2 * half, :], op=mybir.AluOpType.add,
        )
    cur = half
    while cur > 1:
        h2 = cur // 2
        nc.vector.tensor_tensor(
            out=acc[:, 0:h2, :], in0=acc[:, 0:h2, :],
            in1=acc[:, h2:2 * h2, :], op=mybir.AluOpType.add,
        )
        if cur % 2:
            nc.vector.tensor_tensor(
                out=acc[:, 0, :], in0=acc[:, 0, :],
                in1=acc[:, 2 * h2, :], op=mybir.AluOpType.add,
            )
        cur = h2
    return acc


def _build_kernel(Dlo, Dhi, idx_cols):
    nc = bacc.Bacc(None, num_devices=NCORES)

    xT_pos = nc.declare_dram_parameter("xT_pos", [F_IN, NU], bf16, isOutput=False)
    x_ownT = nc.declare_dram_parameter("x_ownT", [F_IN, S], bf16, isOutput=False)
    dinv_pt = nc.declare_dram_parameter("dinv_pt", [P, TPC], f32, isOutput=False)
    idx_in = nc.declare_dram_parameter("idx_in", [P, idx_cols], i16, isOutput=False)
    wg_aug = nc.declare_dram_parameter("wg_aug", [F_IN, TABW], bf16, isOutput=False)
    wg_ad = nc.declare_dram_parameter("wg_ad", [F_IN, H], bf16, isOutput=False)
    wc = nc.declare_dram_parameter("wc", [HC, HID], bf16, isOutput=False)
    wl = nc.declare_dram_parameter("wl", [HID, NCLASS], bf16, isOutput=False)
    bg_b = nc.declare_dram_parameter("bg_b", [P, HC], f32, isOutput=False)
    bc_b = nc.declare_dram_parameter("bc_b", [P, HID], f32, isOutput=False)
    bl_b = nc.declare_dram_parameter("bl_b", [P, NCLASS], f32, isOutput=False)
    out = nc.declare_dram_parameter("out", [S, NCLASS], f32, isOutput=True)

    h_table = nc.dram_tensor("h_table", [NU, TABW], bf16)
    ag_in = nc.dram_tensor("ag_in", [S, HID], bf16)
    ag_out = nc.dram_tensor("ag_out", [NU, HID], bf16, addr_space="Shared")

    AB = 2                       # phase-A tiles per batch (PSUM-bank aligned)
    NA = NU // (P * AB)          # 196 phase-A iterations
    A2B = 8                      # phase-A2 tiles per batch

    with tile.TileContext(nc) as tc:
        with (
            tc.tile_pool(name="const", bufs=1) as cpool,
            tc.tile_pool(name="sa", bufs=3) as sa,
            tc.tile_pool(name="gat", bufs=2) as gp,
            tc.tile_pool(name="scratch", bufs=1) as sp,
            tc.tile_pool(name="ps_a", bufs=2, space="PSUM") as pp_a,
            tc.tile_pool(name="ps_c", bufs=2, space="PSUM") as pp_c,
            tc.tile_pool(name="ps_f", bufs=2, space="PSUM") as pp_f,
        ):
            # ---- resident constants ----
            wga_t = cpool.tile([F_IN, TABW], bf16)
            nc.sync.dma_start(out=wga_t[:], in_=wg_aug[:])
            wgad_t = cpool.tile([F_IN, H], bf16)
            nc.sync.dma_start(out=wgad_t[:], in_=wg_ad[:])
            wc_t = cpool.tile([P, 2, HID], bf16)
            nc.sync.dma_start(out=wc_t[:], in_=wc.rearrange("(k p) n -> p k n", p=P))
            wl_t = cpool.tile([HID, NCLASS], bf16)
            nc.sync.dma_start(out=wl_t[:], in_=wl[:])
            bg_t = cpool.tile([P, HC], f32)
            nc.sync.dma_start(out=bg_t[:], in_=bg_b[:])
            bc_t = cpool.tile([P, HID], f32)
            nc.sync.dma_start(out=bc_t[:], in_=bc_b[:])
            bl_t = cpool.tile([P, NCLASS], f32)
            nc.sync.dma_start(out=bl_t[:], in_=bl_b[:])
            dinv_t = cpool.tile([P, TPC], f32)
            nc.sync.dma_start(out=dinv_t[:], in_=dinv_pt[:])
            idx_t = cpool.tile([P, idx_cols], i16)
            nc.sync.dma_start(out=idx_t[:], in_=idx_in[:])
            ad_all = cpool.tile([P, TPC * H], f32)

            # ---- phase A: build h table (batched 4 node-tiles) ----
            for i in range(NA):
                xt = sa.tile([F_IN, AB * P], bf16, tag="xa")
                nc.sync.dma_start(
                    out=xt[:], in_=xT_pos[:, i * AB * P:(i + 1) * AB * P]
                )
                hps = pp_a.tile([P, AB, 512], f32, tag="a_ps")
                for j in range(AB):
                    nc.tensor.matmul(
                        hps[:, j, 0:TABW], lhsT=xt[:, j * P:(j + 1) * P],
                        rhs=wga_t[:], start=True, stop=True,
                    )
                hbf = sa.tile([P, AB, TABW], bf16, tag="hbf")
                nc.vector.tensor_copy(out=hbf[:], in_=hps[:, :, 0:TABW])
                nc.sync.dma_start(
                    out=h_table[i * AB * P:(i + 1) * AB * P, :].rearrange(
                        "(a p) w -> p a w", p=P
                    ),
                    in_=hbf[:],
                )

            # ---- patch a_s of the two dummy rows ----
            dum = cpool.tile([1, H], bf16)
            nc.vector.memset(dum[:], ASD_NEG)
            nc.sync.dma_start(out=h_table[LO_DUMMY:LO_DUMMY + 1, HC:HC + H], in_=dum[:])
            nc.sync.dma_start(out=h_table[HI_DUMMY:HI_DUMMY + 1, HC:HC + H], in_=dum[:])

            # ---- phase A2: a_d for this core's own nodes ----
            for i in range(TPC // A2B + 1):
                t0 = i * A2B
                nt = min(A2B, TPC - t0)
                if nt <= 0:
                    break
                xt = sa.tile([F_IN, A2B * P], bf16, tag="xa2")
                nc.sync.dma_start(
                    out=xt[:, 0:nt * P], in_=x_ownT[:, t0 * P:(t0 + nt) * P]
                )
                cps = pp_c.tile([P, HID], f32, tag="c_ps")
                adps = cps[:, 0:A2B * H].rearrange("p (a h) -> p a h", a=A2B)
                for j in range(nt):
                    nc.tensor.matmul(
                        adps[:, j, :], lhsT=xt[:, j * P:(j + 1) * P], rhs=wgad_t[:],
                        start=True, stop=True,
                    )
                nc.vector.tensor_copy(
                    out=ad_all[:, t0 * H:(t0 + nt) * H], in_=adps[:, 0:nt, :]
                )

            # ---- phase B+C: GAT + u production per tile ----
            goff = 0
            for t in range(TPC):
                dlo, dhi = Dlo[t], Dhi[t]
                D = dlo + dhi
                G = gp.tile([P, D, TABW], bf16, tag="G")
                if dlo > 0:
                    nc.gpsimd.dma_gather(
                        out_ap=G[:, 0:dlo, :],
                        in_ap=h_table[:, :],
                        idxs_ap=idx_t[:, goff:goff + 8 * dlo],
                        num_idxs=P * dlo,
                        num_idxs_reg=P * dlo,
                        elem_size=TABW,
                        single_packet=False,
                    )
                if dhi > 0:
                    nc.gpsimd.dma_gather(
                        out_ap=G[:, dlo:D, :],
                        in_ap=h_table[SPLIT_AT:, :],
                        idxs_ap=idx_t[:, goff + 8 * dlo:goff + 8 * D],
                        num_idxs=P * dhi,
                        num_idxs_reg=P * dhi,
                        elem_size=TABW,
                        single_packet=False,
                    )
                goff += 8 * D

                # e = a_s[src] + a_d[dst]  -> [P, H, D]
                e = sp.tile([P, H, D], f32, tag="e")
                nc.vector.tensor_tensor(
                    out=e[:],
                    in0=G[:, :, HC:HC + H].rearrange("p d h -> p h d"),
                    in1=ad_all[:, t * H:(t + 1) * H][:, :, None].to_broadcast(
                        [P, H, D]
                    ),
                    op=mybir.AluOpType.add,
                )
                # leaky relu
                e2 = sp.tile([P, H, D], f32, tag="e2")
                nc.vector.tensor_scalar(
                    out=e2[:], in0=e[:], scalar1=NEG, scalar2=None,
                    op0=mybir.AluOpType.mult,
                )
                nc.vector.tensor_tensor(
                    out=e2[:], in0=e[:], in1=e2[:], op=mybir.AluOpType.max
                )
                # softmax over slots
                negm = gp.tile([P, H], f32, tag="negm")
                nc.vector.tensor_reduce(
                    out=negm[:], in_=e2[:], axis=mybir.AxisListType.X,
                    op=mybir.AluOpType.max, negate=True,
                )
                nc.vector.tensor_tensor(
                    out=e2[:], in0=e2[:],
                    in1=negm[:, :, None].to_broadcast([P, H, D]),
                    op=mybir.AluOpType.add,
                )
                ex = sp.tile([P, H, D], f32, tag="ex")
                nc.scalar.activation(
                    out=ex[:], in_=e2[:], func=mybir.ActivationFunctionType.Exp
                )
                den = gp.tile([P, H], f32, tag="den")
                nc.vector.tensor_reduce(
                    out=den[:], in_=ex[:], axis=mybir.AxisListType.X,
                    op=mybir.AluOpType.add,
                )
                rden = gp.tile([P, H], f32, tag="rden")
                nc.vector.reciprocal(rden[:], den[:])
                exn = sp.tile([P, H, D], bf16, tag="exn")
                nc.vector.tensor_tensor(
                    out=exn[:], in0=ex[:],
                    in1=rden[:, :, None].to_broadcast([P, H, D]),
                    op=mybir.AluOpType.mult,
                )
                # prod[p, d, h, c] = h_gathered * alpha
                prod = sp.tile([P, D, HC], bf16, tag="prod")
                nc.vector.tensor_tensor(
                    out=prod.rearrange("p d (h c) -> p d h c", h=H),
                    in0=G[:, :, 0:HC].rearrange("p d (h c) -> p d h c", h=H),
                    in1=exn.rearrange("p h d -> p d h")[:, :, :, None].to_broadcast(
                        [P, D, H, C]
                    ),
                    op=mybir.AluOpType.mult,
                )
                acc = _tree_reduce(nc, sp, prod, D, HC, "accT")
                # xg = elu(acc + bg)
                xg = gp.tile([P, HC], f32, tag="xg")
                nc.vector.tensor_tensor(
                    out=xg[:], in0=acc[:, 0, :], in1=bg_t[:], op=mybir.AluOpType.add
                )
                xn = gp.tile([P, HC], f32, tag="xn")
                nc.vector.tensor_scalar(
                    out=xn[:], in0=xg[:], scalar1=0.0, scalar2=None,
                    op0=mybir.AluOpType.min,
                )
                nc.scalar.activation(
                    out=xn[:], in_=xn[:], func=mybir.ActivationFunctionType.Exp
                )
                nc.vector.tensor_scalar(
                    out=xg[:], in0=xg[:], scalar1=0.0, scalar2=None,
                    op0=mybir.AluOpType.max,
                )
                xgb = gp.tile([P, HC], bf16, tag="xgb")
                nc.vector.tensor_tensor(
                    out=xg[:], in0=xg[:], in1=xn[:], op=mybir.AluOpType.add
                )
                nc.vector.tensor_scalar(
                    out=xgb[:], in0=xg[:], scalar1=-1.0, scalar2=None,
                    op0=mybir.AluOpType.add,
                )
                # phase C: u = dinv * (xgb @ Wc)
                xwps = pp_c.tile([P, HID], f32, tag="c_ps")
                for k in range(2):
                    xgT = sa.tile([P, P], bf16, tag="xgT")
                    nc.sync.dma_start(
                        out=xgT[:], in_=xgb[:, k * P:(k + 1) * P], transpose=True
                    )
                    nc.tensor.matmul(
                        xwps[:], lhsT=xgT[:], rhs=wc_t[:, k, :],
                        start=(k == 0), stop=(k == 1),
                    )
                ub = gp.tile([P, HID], bf16, tag="ub")
                nc.vector.tensor_scalar(
                    out=ub[:], in0=xwps[:], scalar1=dinv_t[:, t:t + 1], scalar2=None,
                    op0=mybir.AluOpType.mult,
                )
                nc.sync.dma_start(out=ag_in[t * P:(t + 1) * P, :], in_=ub[:])

            # ---- phase D: AllGather u across cores ----
            nc.gpsimd.collective_compute(
                "AllGather",
                mybir.AluOpType.bypass,
                replica_groups=[list(range(NCORES))],
                ins=[ag_in[:]],
                outs=[ag_out[:]],
            )

            # ---- phase E+F: GCN + classifier ----
            goff = 0
            lgps = None
            for t in range(TPC):
                dlo, dhi = Dlo[t], Dhi[t]
                D = dlo + dhi
                Gu = gp.tile([P, D, HID], bf16, tag="Gu")
                if dlo > 0:
                    nc.gpsimd.dma_gather(
                        out_ap=Gu[:, 0:dlo, :],
                        in_ap=ag_out[:, :],
                        idxs_ap=idx_t[:, goff:goff + 8 * dlo],
                        num_idxs=P * dlo,
                        num_idxs_reg=P * dlo,
                        elem_size=HID,
                        single_packet=False,
                    )
                if dhi > 0:
                    nc.gpsimd.dma_gather(
                        out_ap=Gu[:, dlo:D, :],
                        in_ap=ag_out[SPLIT_AT:, :],
                        idxs_ap=idx_t[:, goff + 8 * dlo:goff + 8 * D],
                        num_idxs=P * dhi,
                        num_idxs_reg=P * dhi,
                        elem_size=HID,
                        single_packet=False,
                    )
                goff += 8 * D
                uacc = _tree_reduce(nc, sp, Gu, D, HID, "uaccT")
                # xc = relu(dinv * sum + bc), cast bf16
                xc = gp.tile([P, HID], f32, tag="xc")
                nc.vector.tensor_scalar(
                    out=xc[:], in0=uacc[:, 0, :], scalar1=dinv_t[:, t:t + 1],
                    scalar2=None, op0=mybir.AluOpType.mult,
                )
                nc.vector.tensor_tensor(
                    out=xc[:], in0=xc[:], in1=bc_t[:], op=mybir.AluOpType.add
                )
                xcb = gp.tile([P, HID], bf16, tag="xcb")
                nc.vector.tensor_scalar(
                    out=xcb[:], in0=xc[:], scalar1=0.0, scalar2=None,
                    op0=mybir.AluOpType.max,
                )
                # classifier matmul into the group PSUM
                g = t % FGRP
                if g == 0:
                    ngrp = min(FGRP, TPC - t)
                    lgps = pp_f.tile([P, FGRP, NCLASS], f32, tag="f_ps")
                xcT = sa.tile([P, P], bf16, tag="xcT")
                nc.sync.dma_start(out=xcT[:], in_=xcb[:], transpose=True)
                nc.tensor.matmul(
                    lgps[:, g, :], lhsT=xcT[:], rhs=wl_t[:], start=True, stop=True
                )
                if g == ngrp - 1:
                    t0 = t - g
                    lg = gp.tile([P, FGRP, NCLASS], f32, tag="lg")
                    nc.vector.tensor_tensor(
                        out=lg[:, 0:ngrp, :], in0=lgps[:, 0:ngrp, :],
                        in1=bl_t[:, None, :].to_broadcast([P, ngrp, NCLASS]),
                        op=mybir.AluOpType.add,
                    )
                    nmx = gp.tile([P, FGRP], f32, tag="nmx")
                    nc.vector.tensor_reduce(
                        out=nmx[:, 0:ngrp], in_=lg[:, 0:ngrp, :],
                        axis=mybir.AxisListType.X,
                        op=mybir.AluOpType.max, negate=True,
                    )
                    nc.vector.tensor_tensor(
                        out=lg[:, 0:ngrp, :], in0=lg[:, 0:ngrp, :],
                        in1=nmx[:, 0:ngrp, None].to_broadcast([P, ngrp, NCLASS]),
                        op=mybir.AluOpType.add,
                    )
                    exl = gp.tile([P, FGRP, NCLASS], f32, tag="exl")
                    nc.scalar.activation(
                        out=exl[:, 0:ngrp, :], in_=lg[:, 0:ngrp, :],
                        func=mybir.ActivationFunctionType.Exp,
                    )
                    sume = gp.tile([P, FGRP], f32, tag="sume")
                    nc.vector.tensor_reduce(
                        out=sume[:, 0:ngrp], in_=exl[:, 0:ngrp, :],
                        axis=mybir.AxisListType.X, op=mybir.AluOpType.add,
                    )
                    lns = gp.tile([P, FGRP], f32, tag="lns")
                    nc.scalar.activation(
                        out=lns[:, 0:ngrp], in_=sume[:, 0:ngrp],
                        func=mybir.ActivationFunctionType.Ln,
                    )
                    ot = gp.tile([P, FGRP, NCLASS], f32, tag="ot")
                    nc.vector.tensor_tensor(
                        out=ot[:, 0:ngrp, :], in0=lg[:, 0:ngrp, :],
                        in1=lns[:, 0:ngrp, None].to_broadcast([P, ngrp, NCLASS]),
                        op=mybir.AluOpType.subtract,
                    )
                    nc.sync.dma_start(
                        out=out[t0 * P:(t0 + ngrp) * P, :].rearrange(
                            "(a p) n -> p a n", p=P
                        ),
                        in_=ot[:, 0:ngrp, :],
                    )

    nc.compile()
    return nc


def _prepare(inputs):
    x = np.asarray(inputs["x"], np.float32)
    Wg = np.asarray(inputs["Wg"], np.float32)
    att_src = np.asarray(inputs["att_src"], np.float32)
    att_dst = np.asarray(inputs["att_dst"], np.float32)
    bg = np.asarray(inputs["bg"], np.float32)
    Wc = np.asarray(inputs["Wc"], np.float32)
    bc = np.asarray(inputs["bc"], np.float32)
    Wl = np.asarray(inputs["Wl"], np.float32)
    bl = np.asarray(inputs["bl"], np.float32)
    edge_index = np.asarray(inputs["edge_index"])

    st = _build_structures(edge_index)

    As = np.zeros((HC, H), np.float32)
    Ad = np.zeros((HC, H), np.float32)
    for h in range(H):
        As[h * C:(h + 1) * C, h] = att_src[h]
        Ad[h * C:(h + 1) * C, h] = att_dst[h]
    wg_aug = np.concatenate(
        [Wg, Wg @ As, np.zeros((F_IN, TABW - HC - H), np.float32)], axis=1
    )
    wg_ad = Wg @ Ad

    bf = ml_dtypes.bfloat16
    # x in pos order, transposed
    x_pos = np.zeros((NU, F_IN), np.float32)
    for c in range(NCORES):
        x_pos[c * S:c * S + NPC] = x[st["perm"][c]]
    xT_pos = np.ascontiguousarray(x_pos.T).astype(bf)

    in_maps = []
    for c in range(NCORES):
        dv = np.zeros((P, TPC), np.float32)
        dvp = np.zeros(S, np.float32)
        dvp[:NPC] = st["dinv"][st["perm"][c]]
        dv[:, :] = dvp.reshape(TPC, P).T
        in_maps.append({
            "xT_pos": xT_pos,
            "x_ownT": np.ascontiguousarray(xT_pos[:, c * S:(c + 1) * S]),
            "dinv_pt": dv,
            "idx_in": st["idx"][c],
            "wg_aug": wg_aug.astype(bf),
            "wg_ad": wg_ad.astype(bf),
            "wc": Wc.astype(bf),
            "wl": Wl.astype(bf),
            "bg_b": np.tile(bg[None, :], (P, 1)),
            "bc_b": np.tile(bc[None, :], (P, 1)),
            "bl_b": np.tile(bl[None, :], (P, 1)),
        })
    return st, in_maps


def _run(inputs, trace=False, trace_kwargs=None):
    st, in_maps = _prepare(inputs)
    nc = _build_kernel(st["Dlo"], st["Dhi"], st["idx"][0].shape[1])
    res = run_bass_kernel_spmd(
        nc, in_maps, list(range(NCORES)), trace=trace, **(trace_kwargs or {})
    )
    out = np.empty((N, NCLASS), np.float32)
    for c in range(NCORES):
        out[st["perm"][c]] = res.results[c]["out"][:NPC]
    return out, res


def kernel(**inputs) -> np.ndarray:
    out, _ = _run(inputs, trace=False)
    return out


# revision 17
# speedup vs baseline: 1.5317x; 1.1773x over previous
"""GAT + GCN + classifier over a COO graph, distributed over 8 TRN2 NeuronCores.

v2 strategy (dst-sharded message passing, shared pos-ordered tables):
  - Nodes are dealt to 8 cores by degree, then re-dealt within the lo group
    (cores 0-4) and hi group (cores 5-7) so both phases' gather tables share
    ONE row order: table row of a node is its slot position `pos`.  The int16
    gather-index split is at row 31360 (= core 5 start), so lo/hi membership
    is "source core < 5" for BOTH the GAT h-table and the GCN u-table --
    one index array serves both phases.
  - Within a core, nodes are sorted by (d_lo, d_hi) with a 4-tile windowed
    re-sort by d_hi: per-tile padded slot count is ~1.25x the true edge count.
  - x arrives pre-transposed and pos-ordered; phase A is a pure
    load->matmul->cast->store pipeline (batched 4 tiles) building the bf16
    h-table [row = h(256) | a_s(4) | pad], plus per-core a_d columns.
  - GAT per dst tile: 2 dma_gathers (lo/hi), softmax fused as a handful of
    whole-tile strided-AP vector ops, weighted sum via broadcast multiply +
    pairwise tree reduce, ELU, then u = dinv*(xg @ Wc) via DMA-transpose +
    matmul, stored to ag_in.
  - AllGather u across cores; GCN per tile: 2 dma_gathers from the u table,
    tree reduce, relu epilogue, classifier matmul batched 4 tiles per PSUM
    group with one fused log_softmax chain per group.
"""
import sys

sys.path.insert(0, "/opt/trn_rl_repo")

import numpy as np
import ml_dtypes

import concourse.bass as bass
import concourse.bacc as bacc
import concourse.mybir as mybir
import concourse.tile as tile
from concourse.bass_utils import run_bass_kernel_spmd

# problem constants (hardcoded per contract)
N = 50000
E = 800000
F_IN = 128
H = 4
C = 64
HC = H * C          # 256
HID = 128
NCLASS = 10
NEG = 0.2

NCORES = 8
P = 128
NPC = N // NCORES   # 6250 real nodes per core
TPC = 49            # tiles per core
S = TPC * P         # 6272 slots per core
NU = NCORES * S     # 50176 table rows (pos space)
NLO = 5             # cores 0..4 are the "lo" half
SPLIT_AT = NLO * S  # 31360; hi rows span [31360, 50176) -> fits int16 shifted
TABW = 384          # h-table row: 0:256 h | 256:260 a_s | 260:384 zero pad
ASD_NEG = -10000.0
LO_DUMMY = 6271           # pos of a core-0 pad slot (< SPLIT_AT)
HI_DUMMY = NU - 1         # pos of a core-7 pad slot (>= SPLIT_AT)

f32 = mybir.dt.float32
bf16 = mybir.dt.bfloat16
i16 = mybir.dt.int16

FGRP = 4            # classifier tiles per PSUM/log-softmax group


def _build_structures(edge_index):
    src = np.asarray(edge_index[0], dtype=np.int64)
    dst = np.asarray(edge_index[1], dtype=np.int64)
    src = np.concatenate([src, np.arange(N, dtype=np.int64)])
    dst = np.concatenate([dst, np.arange(N, dtype=np.int64)])
    deg = np.bincount(dst, minlength=N).astype(np.int64)
    dinv = (1.0 / np.sqrt(deg)).astype(np.float32)

    # --- deal pass 1: by degree ---
    order = np.argsort(-deg, kind="stable")
    perm = np.stack([order[c::NCORES] for c in range(NCORES)])

    # membership: hi iff source core >= NLO
    core_of = np.empty(N, np.int64)
    for c in range(NCORES):
        core_of[perm[c]] = c
    hi_node = core_of >= NLO
    d_hi = np.bincount(dst[hi_node[src]], minlength=N).astype(np.int64)
    d_lo = deg - d_hi

    # --- deal pass 2 within groups (preserves membership) ---
    key = d_lo * 1000 + d_hi
    lo_nodes = np.where(~hi_node)[0]
    hi_nodes = np.where(hi_node)[0]
    lo_sorted = lo_nodes[np.argsort(-key[lo_nodes], kind="stable")]
    hi_sorted = hi_nodes[np.argsort(-key[hi_nodes], kind="stable")]
    perm = np.empty((NCORES, NPC), np.int64)
    for c in range(NLO):
        perm[c] = lo_sorted[c::NLO]
    for c in range(NCORES - NLO):
        perm[NLO + c] = hi_sorted[c::NCORES - NLO]

    # --- within-core sort: (d_lo, d_hi) then 4-tile window re-sort by d_hi ---
    W = 4 * P
    for c in range(NCORES):
        nodes = perm[c]
        nodes = nodes[np.argsort(-key[nodes], kind="stable")]
        blocks = []
        for s0 in range(0, NPC, W):
            blk = nodes[s0:s0 + W]
            blocks.append(blk[np.argsort(-d_hi[blk], kind="stable")])
        perm[c] = np.concatenate(blocks)

    pos = np.full(N, -1, np.int64)
    for c in range(NCORES):
        pos[perm[c]] = c * S + np.arange(NPC)

    # --- per-tile profiles (max over cores and lanes) ---
    mlo = np.zeros((NCORES, S), np.int64)
    mhi = np.zeros((NCORES, S), np.int64)
    for c in range(NCORES):
        mlo[c, :NPC] = d_lo[perm[c]]
        mhi[c, :NPC] = d_hi[perm[c]]
    Dlo = mlo.reshape(NCORES, TPC, P).max(axis=(0, 2))
    Dhi = mhi.reshape(NCORES, TPC, P).max(axis=(0, 2))

    # --- adjacency grouped by dst, lo sources first; values are pos[src] ---
    hi_e = hi_node[src]
    order_e = np.lexsort((hi_e, dst))
    adj = pos[src[order_e]]
    indptr = np.zeros(N + 1, np.int64)
    np.cumsum(deg, out=indptr[1:])

    def block(nodes, Dt, dcount, base, shift, dummy):
        if Dt == 0:
            return np.zeros(0, np.int64)
        nv = np.maximum(nodes, 0)
        cnt = np.where(nodes >= 0, dcount[nv], 0)
        sl = np.arange(Dt)
        ei = base[:, None] + sl[None, :]
        valid = sl[None, :] < cnt[:, None]
        vals = np.where(valid, adj[np.where(valid, ei, 0)] + shift, dummy)
        return vals.T.reshape(-1)  # position = slot*128 + lane

    def wrap16(flat):
        arr = flat.reshape(-1, 16).T
        return np.tile(arr, (8, 1))

    idx_maps = []
    for c in range(NCORES):
        nodes_pad = np.full(S, -1, np.int64)
        nodes_pad[:NPC] = perm[c]
        cols = []
        for t in range(TPC):
            nodes = nodes_pad[t * P:(t + 1) * P]
            nv = np.maximum(nodes, 0)
            b_lo = indptr[nv]
            b_hi = indptr[nv] + d_lo[nv]
            lo = block(nodes, Dlo[t], d_lo, b_lo, 0, LO_DUMMY)
            hi = block(nodes, Dhi[t], d_hi, b_hi, -SPLIT_AT, HI_DUMMY - SPLIT_AT)
            assert lo.size == 0 or (0 <= lo.min() and lo.max() < 32768)
            assert hi.size == 0 or (0 <= hi.min() and hi.max() < 32768)
            cols.append(wrap16(lo))
            cols.append(wrap16(hi))
        idx_maps.append(np.concatenate(cols, axis=1).astype(np.int16))

    return dict(
        dinv=dinv, perm=perm, pos=pos,
        Dlo=Dlo.tolist(), Dhi=Dhi.tolist(), idx=idx_maps,
    )


def _tree_reduce(nc, sp, prod, D, width, tag):
    """Pairwise tree sum over the slot axis of prod [P, D, width] -> [P, width] f32."""
    if D == 1:
        acc = sp.tile([P, 1, width], f32, tag=tag)
        nc.vector.tensor_copy(out=acc[:, 0, :], in_=prod[:, 0, :])
        return acc
    half = D // 2
    acc = sp.tile([P, max(half, 1), width], f32, tag=tag)
    nc.vector.tensor_tensor(
        out=acc[:, 0:half, :], in0=prod[:, 0:half, :],
        in1=prod[:, half:2 * half, :], op=mybir.AluOpType.add,
    )
    if D % 2:
        nc.vector.tensor_tensor(
            out=acc[:, 0, :], in0=acc[:, 0, :],
            in1=prod[:, 2 * half, :], op=mybir.AluOpType.add,
        )
    cur = half
    while cur > 1:
        h2 = cur // 2
        nc.vector.tensor_tensor(
            out=acc[:, 0:h2, :], in0=acc[:, 0:h2, :],
            in1=acc[:, h2:2 * h2, :], op=mybir.AluOpType.add,
        )
        if cur % 2:
            nc.vector.tensor_tensor(
                out=acc[:, 0, :], in0=acc[:, 0, :],
                in1=acc[:, 2 * h2, :], op=mybir.AluOpType.add,
            )
        cur = h2
    return acc


def _build_kernel(Dlo, Dhi, idx_cols):
    nc = bacc.Bacc(None, num_devices=NCORES)

    ident_in = nc.declare_dram_parameter("ident_bf", [P, P], bf16, isOutput=False)
    xT_pos = nc.declare_dram_parameter("xT_pos", [F_IN, NU], bf16, isOutput=False)
    x_ownT = nc.declare_dram_parameter("x_ownT", [F_IN, S], bf16, isOutput=False)
    dinv_pt = nc.declare_dram_parameter("dinv_pt", [P, TPC], f32, isOutput=False)
    idx_in = nc.declare_dram_parameter("idx_in", [P, idx_cols], i16, isOutput=False)
    wg_aug = nc.declare_dram_parameter("wg_aug", [F_IN, TABW], bf16, isOutput=False)
    wg_ad = nc.declare_dram_parameter("wg_ad", [F_IN, H], bf16, isOutput=False)
    wc = nc.declare_dram_parameter("wc", [HC, HID], bf16, isOutput=False)
    wl = nc.declare_dram_parameter("wl", [HID, NCLASS], bf16, isOutput=False)
    bg_b = nc.declare_dram_parameter("bg_b", [P, HC], f32, isOutput=False)
    bc_b = nc.declare_dram_parameter("bc_b", [P, HID], f32, isOutput=False)
    bl_b = nc.declare_dram_parameter("bl_b", [P, NCLASS], f32, isOutput=False)
    out = nc.declare_dram_parameter("out", [S, NCLASS], f32, isOutput=True)

    h_table = nc.dram_tensor("h_table", [NU, TABW], bf16)
    ag_in = nc.dram_tensor("ag_in", [S, HID], bf16)
    ag_out = nc.dram_tensor("ag_out", [NU, HID], bf16, addr_space="Shared")

    AB = 2                       # phase-A tiles per batch (PSUM-bank aligned)
    NA = NU // (P * AB)          # 196 phase-A iterations
    A2B = 8                      # phase-A2 tiles per batch

    with tile.TileContext(nc) as tc:
        with (
            tc.tile_pool(name="const", bufs=1) as cpool,
            tc.tile_pool(name="sa", bufs=3) as sa,
            tc.tile_pool(name="gat", bufs=2) as gp,
            tc.tile_pool(name="scratch", bufs=1) as sp,
            tc.tile_pool(name="ps_a", bufs=1, space="PSUM") as pp_a,
            tc.tile_pool(name="ps_c", bufs=2, space="PSUM") as pp_c,
            tc.tile_pool(name="ps_f", bufs=1, space="PSUM") as pp_f,
            tc.tile_pool(name="ps_t", bufs=2, space="PSUM") as pp_t,
        ):
            # ---- resident constants ----
            ident_bf = cpool.tile([P, P], bf16)
            nc.sync.dma_start(out=ident_bf[:], in_=ident_in[:])
            wga_t = cpool.tile([F_IN, TABW], bf16)
            nc.sync.dma_start(out=wga_t[:], in_=wg_aug[:])
            wgad_t = cpool.tile([F_IN, H], bf16)
            nc.sync.dma_start(out=wgad_t[:], in_=wg_ad[:])
            wc_t = cpool.tile([P, 2, HID], bf16)
            nc.sync.dma_start(out=wc_t[:], in_=wc.rearrange("(k p) n -> p k n", p=P))
            wl_t = cpool.tile([HID, NCLASS], bf16)
            nc.sync.dma_start(out=wl_t[:], in_=wl[:])
            bg_t = cpool.tile([P, HC], f32)
            nc.sync.dma_start(out=bg_t[:], in_=bg_b[:])
            bc_t = cpool.tile([P, HID], f32)
            nc.sync.dma_start(out=bc_t[:], in_=bc_b[:])
            bl_t = cpool.tile([P, NCLASS], f32)
            nc.sync.dma_start(out=bl_t[:], in_=bl_b[:])
            dinv_t = cpool.tile([P, TPC], f32)
            nc.sync.dma_start(out=dinv_t[:], in_=dinv_pt[:])
            idx_t = cpool.tile([P, idx_cols], i16)
            nc.sync.dma_start(out=idx_t[:], in_=idx_in[:])
            ad_all = cpool.tile([P, TPC * H], f32)

            # ---- phase A: build h table (4 node-tiles per DMA pair) ----
            for i in range(NU // (P * 4)):
                xt = sa.tile([F_IN, 4 * P], bf16, tag="xa")
                nc.sync.dma_start(
                    out=xt[:], in_=xT_pos[:, i * 4 * P:(i + 1) * 4 * P]
                )
                hbf = sa.tile([P, 4, TABW], bf16, tag="hbf")
                for j in range(4):
                    hps = pp_a.tile([P, TABW], f32, tag=f"a_ps{j % 2}")
                    nc.tensor.matmul(
                        hps[:], lhsT=xt[:, j * P:(j + 1) * P],
                        rhs=wga_t[:], start=True, stop=True,
                    )
                    nc.vector.tensor_copy(out=hbf[:, j, :], in_=hps[:])
                nc.sync.dma_start(
                    out=h_table[i * 4 * P:(i + 1) * 4 * P, :].rearrange(
                        "(a p) w -> p a w", p=P
                    ),
                    in_=hbf[:],
                )

            # ---- patch a_s of the two dummy rows ----
            dum = cpool.tile([1, H], bf16)
            nc.vector.memset(dum[:], ASD_NEG)
            nc.sync.dma_start(out=h_table[LO_DUMMY:LO_DUMMY + 1, HC:HC + H], in_=dum[:])
            nc.sync.dma_start(out=h_table[HI_DUMMY:HI_DUMMY + 1, HC:HC + H], in_=dum[:])

            # ---- phase A2: a_d for this core's own nodes ----
            for i in range(TPC // A2B + 1):
                t0 = i * A2B
                nt = min(A2B, TPC - t0)
                if nt <= 0:
                    break
                xt = sa.tile([F_IN, A2B * P], bf16, tag="xa2")
                nc.sync.dma_start(
                    out=xt[:, 0:nt * P], in_=x_ownT[:, t0 * P:(t0 + nt) * P]
                )
                cps = pp_c.tile([P, HID], f32, tag="c_ps")
                adps = cps[:, 0:A2B * H].rearrange("p (a h) -> p a h", a=A2B)
                for j in range(nt):
                    nc.tensor.matmul(
                        adps[:, j, :], lhsT=xt[:, j * P:(j + 1) * P], rhs=wgad_t[:],
                        start=True, stop=True,
                    )
                nc.vector.tensor_copy(
                    out=ad_all[:, t0 * H:(t0 + nt) * H], in_=adps[:, 0:nt, :]
                )

            # ---- phase B+C: GAT + u production per tile ----
            goff = 0
            for t in range(TPC):
                dlo, dhi = Dlo[t], Dhi[t]
                D = dlo + dhi
                G = gp.tile([P, D, TABW], bf16, tag="G")
                if dlo > 0:
                    nc.gpsimd.dma_gather(
                        out_ap=G[:, 0:dlo, :],
                        in_ap=h_table[:, :],
                        idxs_ap=idx_t[:, goff:goff + 8 * dlo],
                        num_idxs=P * dlo,
                        num_idxs_reg=P * dlo,
                        elem_size=TABW,
                        single_packet=False,
                    )
                if dhi > 0:
                    nc.gpsimd.dma_gather(
                        out_ap=G[:, dlo:D, :],
                        in_ap=h_table[SPLIT_AT:, :],
                        idxs_ap=idx_t[:, goff + 8 * dlo:goff + 8 * D],
                        num_idxs=P * dhi,
                        num_idxs_reg=P * dhi,
                        elem_size=TABW,
                        single_packet=False,
                    )
                goff += 8 * D

                # e = a_s[src] + a_d[dst]  -> [P, H, D]
                e = sp.tile([P, H, D], f32, tag="e")
                nc.vector.tensor_tensor(
                    out=e[:],
                    in0=G[:, :, HC:HC + H].rearrange("p d h -> p h d"),
                    in1=ad_all[:, t * H:(t + 1) * H][:, :, None].to_broadcast(
                        [P, H, D]
                    ),
                    op=mybir.AluOpType.add,
                )
                # leaky relu
                e2 = sp.tile([P, H, D], f32, tag="e2")
                nc.vector.tensor_scalar(
                    out=e2[:], in0=e[:], scalar1=NEG, scalar2=None,
                    op0=mybir.AluOpType.mult,
                )
                nc.vector.tensor_tensor(
                    out=e2[:], in0=e[:], in1=e2[:], op=mybir.AluOpType.max
                )
                # softmax over slots
                negm = gp.tile([P, H], f32, tag="negm")
                nc.vector.tensor_reduce(
                    out=negm[:], in_=e2[:], axis=mybir.AxisListType.X,
                    op=mybir.AluOpType.max, negate=True,
                )
                nc.vector.tensor_tensor(
                    out=e2[:], in0=e2[:],
                    in1=negm[:, :, None].to_broadcast([P, H, D]),
                    op=mybir.AluOpType.add,
                )
                ex = sp.tile([P, H, D], f32, tag="ex")
                nc.scalar.activation(
                    out=ex[:], in_=e2[:], func=mybir.ActivationFunctionType.Exp
                )
                den = gp.tile([P, H], f32, tag="den")
                nc.vector.tensor_reduce(
                    out=den[:], in_=ex[:], axis=mybir.AxisListType.X,
                    op=mybir.AluOpType.add,
                )
                rden = gp.tile([P, H], f32, tag="rden")
                nc.vector.reciprocal(rden[:], den[:])
                exn = sp.tile([P, H, D], bf16, tag="exn")
                nc.vector.tensor_tensor(
                    out=exn[:], in0=ex[:],
                    in1=rden[:, :, None].to_broadcast([P, H, D]),
                    op=mybir.AluOpType.mult,
                )
                # prod[p, d, h, c] = h_gathered * alpha
                prod = sp.tile([P, D, HC], bf16, tag="prod")
                nc.vector.tensor_tensor(
                    out=prod.rearrange("p d (h c) -> p d h c", h=H),
                    in0=G[:, :, 0:HC].rearrange("p d (h c) -> p d h c", h=H),
                    in1=exn.rearrange("p h d -> p d h")[:, :, :, None].to_broadcast(
                        [P, D, H, C]
                    ),
                    op=mybir.AluOpType.mult,
                )
                acc = _tree_reduce(nc, sp, prod, D, HC, "accT")
                # xg = elu(acc + bg)
                xg = gp.tile([P, HC], f32, tag="xg")
                nc.vector.tensor_tensor(
                    out=xg[:], in0=acc[:, 0, :], in1=bg_t[:], op=mybir.AluOpType.add
                )
                xn = gp.tile([P, HC], f32, tag="xn")
                nc.vector.tensor_scalar(
                    out=xn[:], in0=xg[:], scalar1=0.0, scalar2=None,
                    op0=mybir.AluOpType.min,
                )
                nc.scalar.activation(
                    out=xn[:], in_=xn[:], func=mybir.ActivationFunctionType.Exp
                )
                nc.vector.tensor_scalar(
                    out=xg[:], in0=xg[:], scalar1=0.0, scalar2=None,
                    op0=mybir.AluOpType.max,
                )
                xgb = gp.tile([P, HC], bf16, tag="xgb")
                nc.vector.tensor_tensor(
                    out=xg[:], in0=xg[:], in1=xn[:], op=mybir.AluOpType.add
                )
                nc.vector.tensor_scalar(
                    out=xgb[:], in0=xg[:], scalar1=-1.0, scalar2=None,
                    op0=mybir.AluOpType.add,
                )
                # phase C: u = dinv * (xgb @ Wc)
                xwps = pp_c.tile([P, HID], f32, tag="c_ps")
                for k in range(2):
                    xgT_ps = pp_t.tile([P, P], bf16, tag="tr_ps")
                    nc.tensor.transpose(
                        xgT_ps[:], xgb[:, k * P:(k + 1) * P], ident_bf[:]
                    )
                    xgT = sa.tile([P, P], bf16, tag="xgT")
                    nc.vector.tensor_copy(out=xgT[:], in_=xgT_ps[:])
                    nc.tensor.matmul(
                        xwps[:], lhsT=xgT[:], rhs=wc_t[:, k, :],
                        start=(k == 0), stop=(k == 1),
                    )
                ub = gp.tile([P, HID], bf16, tag="ub")
                nc.vector.tensor_scalar(
                    out=ub[:], in0=xwps[:], scalar1=dinv_t[:, t:t + 1], scalar2=None,
                    op0=mybir.AluOpType.mult,
                )
                nc.sync.dma_start(out=ag_in[t * P:(t + 1) * P, :], in_=ub[:])

            # ---- phase D: AllGather u across cores ----
            nc.gpsimd.collective_compute(
                "AllGather",
                mybir.AluOpType.bypass,
                replica_groups=[list(range(NCORES))],
                ins=[ag_in[:]],
                outs=[ag_out[:]],
            )

            # ---- phase E+F: GCN + classifier ----
            goff = 0
            lgps = None
            for t in range(TPC):
                dlo, dhi = Dlo[t], Dhi[t]
                D = dlo + dhi
                Gu = gp.tile([P, D, HID], bf16, tag="Gu")
                if dlo > 0:
                    nc.gpsimd.dma_gather(
                        out_ap=Gu[:, 0:dlo, :],
                        in_ap=ag_out[:, :],
                        idxs_ap=idx_t[:, goff:goff + 8 * dlo],
                        num_idxs=P * dlo,
                        num_idxs_reg=P * dlo,
                        elem_size=HID,
                        single_packet=False,
                    )
                if dhi > 0:
                    nc.gpsimd.dma_gather(
                        out_ap=Gu[:, dlo:D, :],
                        in_ap=ag_out[SPLIT_AT:, :],
                        idxs_ap=idx_t[:, goff + 8 * dlo:goff + 8 * D],
                        num_idxs=P * dhi,
                        num_idxs_reg=P * dhi,
                        elem_size=HID,
                        single_packet=False,
                    )
                goff += 8 * D
                uacc = _tree_reduce(nc, sp, Gu, D, HID, "uaccT")
                # xc = relu(dinv * sum + bc), cast bf16
                xc = gp.tile([P, HID], f32, tag="xc")
                nc.vector.tensor_scalar(
                    out=xc[:], in0=uacc[:, 0, :], scalar1=dinv_t[:, t:t + 1],
                    scalar2=None, op0=mybir.AluOpType.mult,
                )
                nc.vector.tensor_tensor(
                    out=xc[:], in0=xc[:], in1=bc_t[:], op=mybir.AluOpType.add
                )
                xcb = gp.tile([P, HID], bf16, tag="xcb")
                nc.vector.tensor_scalar(
                    out=xcb[:], in0=xc[:], scalar1=0.0, scalar2=None,
                    op0=mybir.AluOpType.max,
                )
                # classifier matmul into the group PSUM
                g = t % FGRP
                if g == 0:
                    ngrp = min(FGRP, TPC - t)
                    lgps = pp_f.tile([P, FGRP, NCLASS], f32, tag="f_ps")
                xcT_ps = pp_t.tile([P, P], bf16, tag="tr_ps")
                nc.tensor.transpose(xcT_ps[:], xcb[:], ident_bf[:])
                xcT = sa.tile([P, P], bf16, tag="xcT")
                nc.vector.tensor_copy(out=xcT[:], in_=xcT_ps[:])
                nc.tensor.matmul(
                    lgps[:, g, :], lhsT=xcT[:], rhs=wl_t[:], start=True, stop=True
                )
                if g == ngrp - 1:
                    t0 = t - g
                    lg = gp.tile([P, FGRP, NCLASS], f32, tag="lg")
                    nc.vector.tensor_tensor(
                        out=lg[:, 0:ngrp, :], in0=lgps[:, 0:ngrp, :],
                        in1=bl_t[:, None, :].to_broadcast([P, ngrp, NCLASS]),
                        op=mybir.AluOpType.add,
                    )
                    nmx = gp.tile([P, FGRP], f32, tag="nmx")
                    nc.vector.tensor_reduce(
                        out=nmx[:, 0:ngrp], in_=lg[:, 0:ngrp, :],
                        axis=mybir.AxisListType.X,
                        op=mybir.AluOpType.max, negate=True,
                    )
                    nc.vector.tensor_tensor(
                        out=lg[:, 0:ngrp, :], in0=lg[:, 0:ngrp, :],
                        in1=nmx[:, 0:ngrp, None].to_broadcast([P, ngrp, NCLASS]),
                        op=mybir.AluOpType.add,
                    )
                    exl = gp.tile([P, FGRP, NCLASS], f32, tag="exl")
                    nc.scalar.activation(
                        out=exl[:, 0:ngrp, :], in_=lg[:, 0:ngrp, :],
                        func=mybir.ActivationFunctionType.Exp,
                    )
                    sume = gp.tile([P, FGRP], f32, tag="sume")
                    nc.vector.tensor_reduce(
                        out=sume[:, 0:ngrp], in_=exl[:, 0:ngrp, :],
                        axis=mybir.AxisListType.X, op=mybir.AluOpType.add,
                    )
                    lns = gp.tile([P, FGRP], f32, tag="lns")
                    nc.scalar.activation(
                        out=lns[:, 0:ngrp], in_=sume[:, 0:ngrp],
                        func=mybir.ActivationFunctionType.Ln,
                    )
                    ot = gp.tile([P, FGRP, NCLASS], f32, tag="ot")
                    nc.vector.tensor_tensor(
                        out=ot[:, 0:ngrp, :], in0=lg[:, 0:ngrp, :],
                        in1=lns[:, 0:ngrp, None].to_broadcast([P, ngrp, NCLASS]),
                        op=mybir.AluOpType.subtract,
                    )
                    nc.sync.dma_start(
                        out=out[t0 * P:(t0 + ngrp) * P, :].rearrange(
                            "(a p) n -> p a n", p=P
                        ),
                        in_=ot[:, 0:ngrp, :],
                    )

    nc.compile()
    return nc


def _prepare(inputs):
    x = np.asarray(inputs["x"], np.float32)
    Wg = np.asarray(inputs["Wg"], np.float32)
    att_src = np.asarray(inputs["att_src"], np.float32)
    att_dst = np.asarray(inputs["att_dst"], np.float32)
    bg = np.asarray(inputs["bg"], np.float32)
    Wc = np.asarray(inputs["Wc"], np.float32)
    bc = np.asarray(inputs["bc"], np.float32)
    Wl = np.asarray(inputs["Wl"], np.float32)
    bl = np.asarray(inputs["bl"], np.float32)
    edge_index = np.asarray(inputs["edge_index"])

    st = _build_structures(edge_index)

    As = np.zeros((HC, H), np.float32)
    Ad = np.zeros((HC, H), np.float32)
    for h in range(H):
        As[h * C:(h + 1) * C, h] = att_src[h]
        Ad[h * C:(h + 1) * C, h] = att_dst[h]
    wg_aug = np.concatenate(
        [Wg, Wg @ As, np.zeros((F_IN, TABW - HC - H), np.float32)], axis=1
    )
    wg_ad = Wg @ Ad

    bf = ml_dtypes.bfloat16
    # x in pos order, transposed
    x_pos = np.zeros((NU, F_IN), np.float32)
    for c in range(NCORES):
        x_pos[c * S:c * S + NPC] = x[st["perm"][c]]
    xT_pos = np.ascontiguousarray(x_pos.T).astype(bf)

    in_maps = []
    for c in range(NCORES):
        dv = np.zeros((P, TPC), np.float32)
        dvp = np.zeros(S, np.float32)
        dvp[:NPC] = st["dinv"][st["perm"][c]]
        dv[:, :] = dvp.reshape(TPC, P).T
        in_maps.append({
            "ident_bf": np.eye(P, dtype=bf),
            "xT_pos": xT_pos,
            "x_ownT": np.ascontiguousarray(xT_pos[:, c * S:(c + 1) * S]),
            "dinv_pt": dv,
            "idx_in": st["idx"][c],
            "wg_aug": wg_aug.astype(bf),
            "wg_ad": wg_ad.astype(bf),
            "wc": Wc.astype(bf),
            "wl": Wl.astype(bf),
            "bg_b": np.tile(bg[None, :], (P, 1)),
            "bc_b": np.tile(bc[None, :], (P, 1)),
            "bl_b": np.tile(bl[None, :], (P, 1)),
        })
    return st, in_maps


def _run(inputs, trace=False, trace_kwargs=None):
    st, in_maps = _prepare(inputs)
    nc = _build_kernel(st["Dlo"], st["Dhi"], st["idx"][0].shape[1])
    res = run_bass_kernel_spmd(
        nc, in_maps, list(range(NCORES)), trace=trace, **(trace_kwargs or {})
    )
    out = np.empty((N, NCLASS), np.float32)
    for c in range(NCORES):
        out[st["perm"][c]] = res.results[c]["out"][:NPC]
    return out, res


def kernel(**inputs) -> np.ndarray:
    out, _ = _run(inputs, trace=False)
    return out
